# revision 52
# baseline (speedup 1.0000x reference)
"""Multi-head attention (B=4, S=2048, D=1024, H=16, DK=64) on 8 TRN2 cores.

Sharding: core c = (b, g) with b = c//2 in 0..3 (data parallel on batch) and
g = c%2 (tensor parallel on heads: 8 heads / 512 d' columns per group).
Each core computes a partial output projection; the host sums the two
partials per batch (the "all-reduce" of the sharding hint, done host-side)
and adds bo.

Per-core device algorithm (all matmul inputs bf16, fp32 PSUM accumulation):
  QT[d',q] = (Wq_g^T Xq^T + bq 1^T)     via lhsT=Wq tiles, rhs=XqT tiles
  KT[d',k] = same with Wk
  Vn[k,d'] = Xv Wv_g + 1 bv^T           natural layout, plus a ones column
                                        per head -> V_aug [k, 65] per head
  per (head pair, q-chunk) unit, per k-tile kk:
    ST[k,q]   = scores via lhsT=KT slice, rhs=QT slice (2 row-packed MMs)
    P = exp(ST/8)                       one ACT pass per k-tile (PSUM->SBUF)
    AT_aug    = sum_k V_aug^T P         -> [65, q]: rows 0..63 = V^T P,
                                           row 64 = softmax denominators
  r = 1/den, broadcast via DRAM-bounce DMA, gpsimd multiply -> normalized
  out[q,e] += sum_h ATn_h^T Wo_h        partial output projection (fp32 out)

Schedule (v2): the attention units iterate PAIR-OUTER, q-chunk inner.  Only
pair 0's K/V/Q projections run before the first score matmul (~22us instead
of the full 75us projection stage); every later projection chain and all
output projections are drip-fed into the per-k-tile PE gaps of the
exp-paced attention pipeline (ACT exp = 1085ns per k-tile is the pacing
engine; the PE needs only ~650-900ns per k-tile for scores+AV).
"""

import os
import sys
import time
import types

sys.path.insert(0, "/opt/trn_rl_repo")

import numpy as np
import ml_dtypes

# ---------------------------------------------------------------------------
# axon NTFF profile hook (missing from this image's antenv stub); harmless
# when tracing is disabled.
# ---------------------------------------------------------------------------
def _install_axon_hooks():
    import antenv

    if "antenv.axon_hooks" in sys.modules:
        return
    hooks = types.ModuleType("antenv.axon_hooks")
    hooks._hook = None
    hooks.set_axon_ntff_profile_hook = lambda h: setattr(hooks, "_hook", h)
    hooks.get_axon_ntff_profile_hook = lambda: hooks._hook
    sys.modules["antenv.axon_hooks"] = hooks
    antenv.axon_hooks = hooks
    try:
        from trn_agent_boot.trn_boot import _ntff_profile_via_ctypes

        hooks.set_axon_ntff_profile_hook(
            _ntff_profile_via_ctypes("/opt/axon/libaxon_pjrt.so")
        )
    except Exception:
        pass


_install_axon_hooks()

import concourse.bacc as bacc
import concourse.bass as bass
import concourse.tile as tile
from concourse import mybir
from concourse import bass_utils
from concourse.bass_utils import run_bass_kernel_spmd

# The trace path uploads artifacts to a network bucket; keep it local.
bass_utils.upload_artifacts = lambda tmpdir: tmpdir

BF16 = mybir.dt.bfloat16
F32 = mybir.dt.float32

# Problem dims (hardcoded per spec)
B, S, D = 4, 2048, 1024
H, DK = 16, 64
N_CORES = 8
HC = H // N_CORES * B  # heads per core = 8  (16 heads / 2 groups)
DPC = HC * DK  # d' columns per core = 512

LAST_EXEC_TIME_NS = None


def build_program(s=S, dm=D, hc=HC, e=D):
    """Build the per-core Bass program. All dims in units of elements."""
    dk = DK
    dpc = hc * dk  # d' per core
    pairs = hc // 2
    dt_n = dm // 128  # d-tiles (contraction tiles for projections)
    st_n = s // 128  # s-tiles = k-tiles
    qc_n = s // 512  # q-chunks
    ec_n = e // 512  # out-proj column chunks

    nc = bacc.Bacc("TRN2", target_bir_lowering=False, debug=False,
                   num_devices=N_CORES)

    xqT = nc.dram_tensor("xqT", [dm, s], BF16, kind="ExternalInput")
    xkT = nc.dram_tensor("xkT", [dm, s], BF16, kind="ExternalInput")
    xvT = nc.dram_tensor("xvT", [dm, s], BF16, kind="ExternalInput")
    wq = nc.dram_tensor("wq", [dm, dpc], BF16, kind="ExternalInput")
    wk = nc.dram_tensor("wk", [dm, dpc], BF16, kind="ExternalInput")
    wv = nc.dram_tensor("wv", [dm, dpc], BF16, kind="ExternalInput")
    wo = nc.dram_tensor("wo", [dpc, e], BF16, kind="ExternalInput")
    bq = nc.dram_tensor("bq", [dpc], F32, kind="ExternalInput")
    bk = nc.dram_tensor("bk", [dpc], F32, kind="ExternalInput")
    bv = nc.dram_tensor("bv", [dpc], BF16, kind="ExternalInput")
    out = nc.dram_tensor("out", [s, e], F32, kind="ExternalOutput")

    with tile.TileContext(nc) as tc:
        with (
            tc.tile_pool(name="singles", bufs=1) as singles,
            tc.tile_pool(name="xin", bufs=1) as xin,
            tc.tile_pool(name="expst", bufs=4) as expst_pool,
            tc.tile_pool(name="small", bufs=2) as small,
            tc.tile_pool(name="small1", bufs=1) as small1,
            tc.tile_pool(name="outsb", bufs=2) as outsb_pool,
            tc.tile_pool(name="ps_sc", bufs=2, space="PSUM") as ps_sc,
            tc.tile_pool(name="ps_at", bufs=4, space="PSUM") as ps_at,
            tc.tile_pool(name="dramb", bufs=4, space="DRAM") as dramb,
        ):
            # ---- persistent SBUF tensors ----
            # qt/kt rotate 2-deep across head pairs (pair-outer schedule)
            qt_sb = singles.tile([128, 2, s], BF16, tag="qt")
            kt_sb = singles.tile([128, 2, s], BF16, tag="kt")
            vn_sb = singles.tile([128, st_n, hc, dk + 1], BF16, tag="vn")
            atn_sb = singles.tile([128, qc_n, pairs, 512], BF16, tag="atn")
            # denominator rows: partition qcc*32 + h (32-aligned blocks so
            # the DVE reciprocal's base partition is legal)
            rs_sb = singles.tile([128, 512], F32, tag="rs")
            wq_sb = singles.tile([128, dt_n, dpc], BF16, tag="wq")
            wk_sb = singles.tile([128, dt_n, dpc], BF16, tag="wk")
            wv_sb = singles.tile([128, dt_n, dpc], BF16, tag="wv")
            wo_sb = singles.tile([128, pairs, e], BF16, tag="wo")
            # biases: bq/bk transposed to per-partition [d' p, pair] so the
            # DVE copy-out can fold the add; bv broadcast to all partitions
            # via a stride-0 DMA (it is added along the free dim of Vn)
            bqT_sb = singles.tile([128, pairs], F32, tag="bqT")
            bkT_sb = singles.tile([128, pairs], F32, tag="bkT")
            bv_bc = singles.tile([128, dpc], BF16, tag="bvbc")
            ones_sb = singles.tile([128, 512], BF16, tag="ones")

            # weights: split per d-tile so the first projection matmuls can
            # start as soon as the first 128 KB lands
            def load_w_split(w_sb, wdram):
                src = wdram.ap().rearrange("(t p) n -> p t n", p=128)
                for t in range(dt_n):
                    nc.sync.dma_start(
                        out=w_sb[:, t, :], in_=src[:, t, :])

            # x staging uses a column-block-major layout [p, block, t, cols];
            # block DMAs are issued interleaved in first-use order so the
            # prologue's dependencies land first.
            nblk = 4

            def make_x(tag):
                return xin.tile([128, nblk, dt_n, s // nblk], BF16, tag=tag,
                                name=tag)

            def load_x_block(x_sb, xdram, j, split=True):
                # always split per d-tile: a monolithic block is 1024
                # descriptors on ONE DMA queue (~47us); 8 pieces spread
                # round-robin across queues land in ~6us
                src = xdram.ap().rearrange("(t p) n -> p t n", p=128)
                jsl = slice(j * s // nblk, (j + 1) * s // nblk)
                for t in range(dt_n):
                    nc.sync.dma_start(
                        out=x_sb[:, j, t, :], in_=src[:, t, jsl])

            def xslice(x_sb, t, lo, width):
                bw = s // nblk
                j, off = lo // bw, lo % bw
                assert off + width <= bw
                return x_sb[:, j, t, off : off + width]

            xk_sb = make_x("xk")
            xv_sb = make_x("xv")
            xq_sb = make_x("xq")
            # Issue order = first-use order: K(0,0) needs wk+xk.b0, Q(0,0)
            # needs wq+xq.b0 (+ the bias tiles for the copy-outs), then V's
            # inputs, then the remaining blocks in (0,0)-streaming order.
            load_w_split(wk_sb, wk)
            load_x_block(xk_sb, xkT, 0, split=True)
            nc.sync.dma_start(
                out=bkT_sb, in_=bk.ap().rearrange("(a p) -> p a", p=128))
            nc.sync.dma_start(
                out=bqT_sb, in_=bq.ap().rearrange("(a p) -> p a", p=128))
            load_w_split(wq_sb, wq)
            load_x_block(xq_sb, xqT, 0, split=True)
            bv_row = bv.ap().rearrange("(o n) -> o n", o=1)
            nc.sync.dma_start(
                out=bv_bc,
                in_=bass.AP(tensor=bv_row.tensor, offset=bv_row.offset,
                            ap=[[0, 128]] + list(bv_row.ap[1:])))
            load_w_split(wv_sb, wv)
            load_x_block(xv_sb, xvT, 0, split=True)
            for j in range(1, nblk):
                load_x_block(xk_sb, xkT, j)
                load_x_block(xv_sb, xvT, j)
            for j in range(1, nblk):
                load_x_block(xq_sb, xqT, j)
            nc.sync.dma_start(
                out=wo_sb, in_=wo.ap().rearrange("(a p) e -> p a e", p=128))
            nc.vector.memset(ones_sb, 1.0)
            # ones column of every V_aug head block
            nc.vector.memset(vn_sb[:, :, :, dk : dk + 1], 1.0)

            # Warm-up exp ACT right away: forces the ~2.7us ACT_TABLE_LOAD to
            # overlap the prologue projections instead of stalling the first
            # attention unit.
            warm_sb = singles.tile([128, 32], F32, tag="warm")
            nc.scalar.activation(
                warm_sb, ones_sb[:, 0:32], mybir.ActivationFunctionType.Exp)

            # ---- projection chain emitters (also usable as generators) ----
            def proj_qk_gen(w_sb, b_sb, x_sb, dst, slot, p, qcc):
                """[d' 128, q 512] projection chain for pair p, chunk qcc.
                b_sb is the transposed bias [128, pairs]; the add is folded
                into the DVE copy-out (per-partition scalar)."""
                ps = ps_at.tile([128, 512], F32, tag="ps")
                for t in range(dt_n):
                    nc.tensor.matmul(
                        ps,
                        w_sb[:, t, p * 128 : (p + 1) * 128],
                        xslice(x_sb, t, qcc * 512, 512),
                        start=(t == 0),
                        stop=(t == dt_n - 1),
                    )
                    yield
                nc.vector.tensor_scalar_add(
                    dst[:, slot, qcc * 512 : (qcc + 1) * 512], ps,
                    b_sb[:, p : p + 1])
                yield

            def proj_v_gen(st, c_lo, c_w):
                """V chain: rows st*128.., d' columns [c_lo, c_lo+c_w).
                The bias add (broadcast bv) is folded into the DVE copy."""
                ps = ps_at.tile([128, c_w], F32, tag="ps")
                nsl = slice(c_lo, c_lo + c_w)
                for t in range(dt_n):
                    nc.tensor.matmul(
                        ps,
                        xslice(xv_sb, t, st * 128, 128),
                        wv_sb[:, t, nsl],
                        start=(t == 0),
                        stop=(t == dt_n - 1),
                    )
                    yield
                nc.vector.tensor_add(
                    vn_sb[:, st, c_lo // dk : (c_lo + c_w) // dk, 0:dk],
                    ps.rearrange("p (h d) -> p h d", d=dk),
                    bv_bc[:, nsl].rearrange("p (h d) -> p h d", d=dk),
                )
                yield

            def outproj_gen(qcc, qt_i, ecc):
                """Out-projection sequence (pair-packed, K=128 per matmul)."""
                esl = slice(ecc * 512, (ecc + 1) * 512)
                q0 = qcc * 4 + qt_i
                o_ps = ps_at.tile([128, 512], F32, tag="ps")
                for p in range(pairs):
                    nc.tensor.matmul(
                        o_ps,
                        atn_sb[:, qcc, p, qt_i * 128 : (qt_i + 1) * 128],
                        wo_sb[:, p, esl],
                        start=(p == 0),
                        stop=(p == pairs - 1),
                    )
                    yield
                o_sb = outsb_pool.tile([128, 512], F32, tag="o")
                nc.vector.tensor_copy(o_sb, o_ps)
                nc.sync.dma_start(
                    out=out.ap()[q0 * 128 : (q0 + 1) * 128, esl], in_=o_sb)
                yield

            class FillerQueue:
                def __init__(self):
                    self.tasks = []  # (gen, deadline or None)

                def add(self, gen, deadline=None):
                    self.tasks.append((gen, deadline))

                def pump(self, n):
                    while n > 0 and self.tasks:
                        try:
                            next(self.tasks[0][0])
                            n -= 1
                        except StopIteration:
                            self.tasks.pop(0)

                def fence(self, key):
                    # complete every task whose deadline is <= key (FIFO order
                    # matches deadline order)
                    while self.tasks and any(
                        dl is not None and dl <= key for _, dl in self.tasks
                    ):
                        self.pump(1000)

                def drain(self):
                    while self.tasks:
                        self.pump(1000)

            fill = FillerQueue()

            # ---- prologue: the minimum for the first score matmul ----
            # K pair0 chunk 0 and Q pair0 chunk 0; all of pair 0's V and
            # remaining K stream into unit (0,0) with per-k-tile deadlines.
            for _ in proj_qk_gen(wk_sb, bkT_sb, xk_sb, kt_sb, 0, 0, 0):
                pass
            for _ in proj_qk_gen(wq_sb, bqT_sb, xq_sb, qt_sb, 0, 0, 0):
                pass
            # enqueue in deadline order (fence() pumps the FIFO head)
            stream0 = [
                (proj_v_gen(st, 0, 128), (0, 0, st)) for st in range(st_n)
            ] + [
                (proj_qk_gen(wk_sb, bkT_sb, xk_sb, kt_sb, 0, 0, kcc),
                 (0, 0, 4 * kcc - 1))
                for kcc in range(1, qc_n)
            ]
            for gen, dl in sorted(stream0, key=lambda x: x[1]):
                fill.add(gen, deadline=dl)

            # ---- softmax normalization helpers ----
            # The reciprocal rows bounce through DRAM so a partition-stride-0
            # DMA can broadcast each [1,512] row to 64 partitions; one
            # combined [128,512] gpsimd multiply then normalizes a whole
            # head pair.
            def emit_pair_mults(rec_dram, qcc, prs):
                for p in prs:
                    bc_sb = small.tile([128, 512], F32, tag="bc")
                    for i, h in enumerate((2 * p, 2 * p + 1)):
                        row = rec_dram[h : h + 1, :]
                        bcast_src = bass.AP(
                            tensor=row.tensor,
                            offset=row.offset,
                            ap=[[0, 64]] + list(row.ap[1:]),
                        )
                        nc.sync.dma_start(
                            out=bc_sb[64 * i : 64 * (i + 1), :], in_=bcast_src)
                    nc.gpsimd.tensor_mul(
                        atn_sb[:, qcc, p, :],
                        atn_sb[:, qcc, p, :],
                        bc_sb,
                    )

            def emit_norm(qcc, prs):
                """Normalize pairs `prs` (= [0..n]) of chunk qcc from its
                rs rows."""
                row_n = 2 * len(prs)
                rec_sb = small.tile([hc, 512], F32, tag="rec")
                nc.vector.reciprocal(
                    out=rec_sb[0:row_n, :],
                    in_=rs_sb[qcc * 32 : qcc * 32 + row_n, :])
                rec_dram = dramb.tile([hc, 512], F32, tag="recd")
                nc.sync.dma_start(
                    out=rec_dram[0:row_n, :], in_=rec_sb[0:row_n, :])
                emit_pair_mults(rec_dram, qcc, prs)

            def emit_norm_p3(qcc):
                """Normalize pair 3 of chunk qcc (den in rs rows qcc*32+0/1,
                written there after the block's pairs-0-2 rows died)."""
                rec_sb = small.tile([hc, 512], F32, tag="rec")
                nc.vector.reciprocal(
                    out=rec_sb[0:2, :],
                    in_=rs_sb[qcc * 32 : qcc * 32 + 2, :])
                rec_dram = dramb.tile([hc, 512], F32, tag="recd")
                nc.sync.dma_start(
                    out=rec_dram[6:8, :], in_=rec_sb[0:2, :])
                emit_pair_mults(rec_dram, qcc, [pairs - 1])

            # ---- filler enqueue schedule, keyed by (pr, qcc) block ----
            def enqueue_for_block(pr, qcc):
                if qcc == 0:
                    # current pair's remaining Q chunks
                    for qq in range(1, qc_n):
                        fill.add(proj_qk_gen(wq_sb, bqT_sb, xq_sb, qt_sb,
                                             pr % 2, pr, qq), deadline=(pr, qq))
                elif qcc == 1 and pr < pairs - 1:
                    # V for the following pair(s): pair 1 at N=128 during
                    # pair 0, pairs 2+3 together at N=256 during pair 1
                    if pr == 0:
                        for st in range(0, st_n, 2):
                            fill.add(proj_v_gen(st, 128, 128), deadline=(1, 0))
                    elif pr == 1:
                        for st in range(0, st_n, 2):
                            fill.add(proj_v_gen(st, 256, 256), deadline=(2, 0))
                    nslot = (pr + 1) % 2
                    for kcc in range(qc_n):
                        fill.add(proj_qk_gen(wk_sb, bkT_sb, xk_sb, kt_sb,
                                             nslot, pr + 1, kcc),
                                 deadline=(pr + 1, 0))
                elif qcc == 2 and pr < pairs - 1:
                    if pr == 0:
                        for st in range(1, st_n, 2):
                            fill.add(proj_v_gen(st, 128, 128), deadline=(1, 0))
                    elif pr == 1:
                        for st in range(1, st_n, 2):
                            fill.add(proj_v_gen(st, 256, 256), deadline=(2, 0))
                    fill.add(proj_qk_gen(wq_sb, bqT_sb, xq_sb, qt_sb,
                                         (pr + 1) % 2, pr + 1, 0),
                             deadline=(pr + 1, 0))

            def enqueue_outproj(qcc):
                # out-projection of q-chunk qcc; its atn tiles were written by
                # all four pairs and normalized by the (deferred) norm(qcc).
                for sq in range(4 * ec_n):
                    fill.add(outproj_gen(qcc, sq // ec_n, sq % ec_n))

            # ---- attention units: pair-outer, q-chunk inner ----
            for pr in range(pairs):
                slot = pr % 2
                for qcc in range(qc_n):
                    fill.fence((pr, qcc))
                    enqueue_for_block(pr, qcc)
                    outproj_ready = None
                    if pr == pairs - 1:
                        # norms run on DVE/DMA/gpsimd (off the PE path).
                        # All pairs-0-2 rows (every q-chunk) are ready once
                        # pair 2 finished, so they all normalize at (3,0);
                        # each pair-3 norm follows its own unit.
                        if qcc == 0:
                            for q in range(qc_n):
                                emit_norm(q, [0, 1, 2])
                        else:
                            emit_norm_p3(qcc - 1)
                            outproj_ready = qcc - 1
                    qsl = slice(qcc * 512, (qcc + 1) * 512)

                    at_A = ps_at.tile([65, 512], F32, tag="ps")
                    at_B = ps_at.tile([65, 512], F32, tag="ps")
                    pipe = None  # (kk, exp_sb) awaiting its AT matmuls

                    def emit_at(kk, e_sb, at_A=at_A, at_B=at_B, pr=pr):
                        nc.tensor.matmul(
                            at_A,
                            vn_sb[:, kk, 2 * pr, :],
                            e_sb[:, 0:512],
                            start=(kk == 0),
                            stop=(kk == st_n - 1),
                        )
                        nc.tensor.matmul(
                            at_B,
                            vn_sb[:, kk, 2 * pr + 1, :],
                            e_sb[:, 512:1024],
                            start=(kk == 0),
                            stop=(kk == st_n - 1),
                        )

                    def emit_scores(kk, slot=slot, qsl=qsl):
                        sc_ps = ps_sc.tile([128, 1024], F32, tag="sc")
                        ksl = slice(kk * 128, (kk + 1) * 128)
                        nc.tensor.matmul(
                            sc_ps[:, 0:512],
                            kt_sb[0:64, slot, ksl],
                            qt_sb[0:64, slot, qsl],
                            start=True,
                            stop=True,
                        )
                        nc.tensor.matmul(
                            sc_ps[:, 512:1024],
                            kt_sb[64:128, slot, ksl],
                            qt_sb[64:128, slot, qsl],
                            start=True,
                            stop=True,
                        )
                        return sc_ps

                    # PE order per exp period: AV(kk-1) first (its input is
                    # ready when exp(kk) starts), then scores(kk+1) so the
                    # next exp is never starved, then fillers.
                    sc_cur = emit_scores(0)
                    for kk in range(st_n):
                        fill.fence((pr, qcc, kk))
                        exp_sb = expst_pool.tile([128, 1024], BF16, tag="e")
                        nc.scalar.activation(
                            exp_sb, sc_cur,
                            mybir.ActivationFunctionType.Exp,
                            scale=1.0 / np.sqrt(dk),
                        )
                        if pipe is not None:
                            emit_at(*pipe)
                        pipe = (kk, exp_sb)
                        fill.pump(1)
                        if kk + 1 < st_n:
                            sc_cur = emit_scores(kk + 1)
                        if kk == 6 and outproj_ready is not None:
                            enqueue_outproj(outproj_ready)
                            outproj_ready = None
                        # pair-3 blocks carry the out-projections: one extra
                        # filler slot per k-tile
                        fill.pump(2 if pr == pairs - 1 and qcc >= 1 else 1)
                    emit_at(*pipe)

                    # denominator rows -> rs.  Pairs 0-2 use block rows
                    # 2*pr, 2*pr+1; pair 3 reuses rows 0-1 (dead after the
                    # (3,0) reciprocals).  For pair 3 they go FIRST: the
                    # pair-3 norm chains gate the out-projections.
                    def emit_den(pr=pr, qcc=qcc, at_A=at_A, at_B=at_B):
                        lo = 0 if pr == pairs - 1 else 2 * pr
                        for i, at_ps in ((0, at_A), (1, at_B)):
                            rs_row = small1.tile([65, 512], F32, tag="rsrow")
                            nc.vector.tensor_copy(
                                rs_row[64:65, :], at_ps[64:65, :])
                            nc.sync.dma_start(
                                out=rs_sb[qcc * 32 + lo + i :
                                          qcc * 32 + lo + i + 1, :],
                                in_=rs_row[64:65, :])

                    if pr == pairs - 1:
                        emit_den()
                    # atn pair tile: head A on partitions 0-63 (direct DVE
                    # copy), head B shifted to 64-127 via SBUF->SBUF DMA
                    nc.vector.tensor_copy(
                        atn_sb[0:64, qcc, pr, :], at_A[0:64, :])
                    btmp = small1.tile([64, 512], BF16, tag="btmp")
                    nc.vector.tensor_copy(btmp, at_B[0:64, :])
                    nc.sync.dma_start(
                        out=atn_sb[64:128, qcc, pr, :], in_=btmp)
                    if pr != pairs - 1:
                        emit_den()

            # ---- tail: last pair's norm + last q-chunk's out-projection ----
            emit_norm_p3(qc_n - 1)
            enqueue_outproj(qc_n - 1)
            fill.drain()

    nc.compile()
    return nc


_PROGRAM_CACHE = {}


def _get_program(key):
    if key not in _PROGRAM_CACHE:
        _PROGRAM_CACHE[key] = build_program(*key)
    return _PROGRAM_CACHE[key]


def kernel(queries, keys, values, Wq, bq, Wk, bk, Wv, bv, Wo, bo):
    global LAST_EXEC_TIME_NS
    bf16 = ml_dtypes.bfloat16

    nc = _get_program((S, D, HC, D))

    xT = {}
    for name, arr in (("q", queries), ("k", keys), ("v", values)):
        xT[name] = [
            np.ascontiguousarray(np.asarray(arr[b]).T).astype(bf16)
            for b in range(B)
        ]
    Wq, Wk, Wv, Wo = (np.asarray(w) for w in (Wq, Wk, Wv, Wo))
    bqv, bkv, bvv = (np.asarray(v) for v in (bq, bk, bv))

    in_maps = []
    for c in range(N_CORES):
        b, g = c // 2, c % 2
        csl = slice(g * DPC, (g + 1) * DPC)
        in_maps.append(
            {
                "xqT": xT["q"][b],
                "xkT": xT["k"][b],
                "xvT": xT["v"][b],
                "wq": np.ascontiguousarray(Wq[:, csl]).astype(bf16),
                "wk": np.ascontiguousarray(Wk[:, csl]).astype(bf16),
                "wv": np.ascontiguousarray(Wv[:, csl]).astype(bf16),
                "wo": np.ascontiguousarray(Wo[csl, :]).astype(bf16),
                "bq": np.ascontiguousarray(bqv[csl]).astype(np.float32),
                "bk": np.ascontiguousarray(bkv[csl]).astype(np.float32),
                "bv": np.ascontiguousarray(bvv[csl]).astype(bf16),
            }
        )

    trace = os.environ.get("KERNEL_TRACE", "0") == "1"
    res = run_bass_kernel_spmd(nc, in_maps, list(range(N_CORES)), trace=trace)
    LAST_EXEC_TIME_NS = res.exec_time_ns

    bo = np.asarray(bo, dtype=np.float32)
    out = np.empty((B, S, D), dtype=np.float32)
    for b in range(B):
        out[b] = res.results[2 * b]["out"] + res.results[2 * b + 1]["out"] + bo
    return out


if __name__ == "__main__":
    rng = np.random.default_rng(0)
    t0 = time.time()
    nc = _get_program((S, D, HC, D))
    print(f"build+compile: {time.time() - t0:.1f}s")
